# revision 22
# baseline (speedup 1.0000x reference)
"""Trainium2 Bass kernel for nn_BasicAttention (dense transformer block).

Strategy (pure data parallel over 8 NeuronCores, batch-sharded):
  per core: B_CORE=256 batches of [N=49, DIM=384].
  Per group of G=10 batches (free-packed width W = G*49):
    1. qkT GEMM   : [512 qk-feats, W]  = wT_qk.T @ xT   (f32r, moving=W)
    2. v GEMM     : [113(2-batch packed m), 512]x2 per batch pair (f32r)
    3. scoresT    : bias pre-seeded into PSUM via identity matmul, then
                    per-(batch,head) fp16 matmuls K=32 packed into PE
                    sub-tiles (auto tile_position from base partitions)
    4. softmax    : exp on ACT (PSUM->SBUF fp16), denominators via
                    ones-matmul on PE, reciprocal on DVE, broadcast of
                    1/s back over m-rows via selector matmul on PE
    5. AV         : fp16 matmuls [49,128]x[49,49] -> avT [128(d), W]
    6. proj       : finalT [384, W] accumulated over 8 head chunks (f32r)
  Host side: weight reordering/transposition, rel-pos bias gather,
  x transposition+padding, output transposition.
"""

import sys

sys.path.insert(0, "/opt/trn_rl_repo")

import numpy as np

import concourse.bass as bass
import concourse.mybir as mybir
import concourse.tile as tile
from concourse.vector_clock import ScopedClock

# ---------------- problem constants (hardcoded per spec) ----------------
B = 2048
N = 49
DIM = 384
H = 8
KD = 32
D = 128
DH = H * D  # 1024
HQKV = 1536
N_CORES = 8
B_CORE = B // N_CORES  # 256
G = 10  # batches per group
NP = 64  # padded token stride (m padded 49->64 inside x tiles)

F32R = mybir.dt.float32r
F32 = mybir.dt.float32
F16 = mybir.dt.float16

_CACHE = {}


# ---------------- Tile drain workaround ----------------
def _patched_drain_and_barrier(self, tick_clock, wait_clock):
    # walrus rejects >1 sem wait on the final SP Drain (TPB_CTRL); spread
    # the global-clock waits across single-wait SP nops instead.
    nc = self.nc
    probe = nc.sync.nop()
    wait_clock.add_sem_waits(probe.ins, ScopedClock({None: tick_clock.global_clock}))
    waits = []
    if probe.ins.sync_info and probe.ins.sync_info.on_wait:
        waits = list(probe.ins.sync_info.on_wait)
        probe.ins.sync_info.on_wait = waits[:1]
    assert self.sems is not None
    handles = list(self.sems.allocated().values())
    for w in waits[1:]:
        n = nc.sync.nop()
        n._wait_ge(handles[0], 0)
        n.ins.sync_info.on_wait = [w]
    nc.sync.drain()
    nc.all_engine_barrier()
    popped = nc._tile_sem_poison_stack.pop()
    assert popped is self._sem_poison
    nc.clear_and_free_semaphores(handles)
    nc.all_engine_barrier()


tile.TileContext._drain_and_barrier = _patched_drain_and_barrier


def _legalize_waits(nc, max_waits=1):
    """walrus on this toolchain rejects instructions carrying more than one
    sem wait; split excess waits onto preceding same-engine nops."""
    import bass_rust

    cnt = 0
    for f in nc.m.functions:
        for b in f.blocks:
            insts = b.instructions
            inserts = []
            for idx, inst in enumerate(insts):
                si = inst.sync_info
                waits = list(si.on_wait) if (si and si.on_wait) else []
                if len(waits) <= max_waits:
                    continue
                nops = []
                for w in waits[max_waits:]:
                    cnt += 1
                    nop = mybir.InstNoOp(
                        name=f"I-waitsplit-{cnt}",
                        engine=inst.engine,
                        ins=[],
                        outs=[],
                        sync_info=bass_rust.SyncInfo(on_wait=[w], on_update=[]),
                    )
                    try:
                        nc.register_instruction(nop)
                    except Exception:
                        pass
                    nops.append(nop)
                si.on_wait = waits[:max_waits]
                inserts.append((idx, nops))
            for idx, nops in reversed(inserts):
                for nop in reversed(nops):
                    insts.insert(idx, nop)
    return cnt


def _group_sizes(b_core=B_CORE):
    sizes = []
    b = 0
    while b < b_core:
        g = min(G, b_core - b)
        sizes.append(g)
        b += g
    return sizes


def _build_bass(with_qkv_bias, with_proj_bias, b_core=B_CORE):
    nc = bass.Bass()

    xt = nc.declare_dram_parameter("xt", [b_core, 3, 128, NP], F32R, isOutput=False)
    wt = nc.declare_dram_parameter("wt", [3, 128, HQKV], F32R, isOutput=False)
    projt = nc.declare_dram_parameter("projt", [128, H, DIM], F32R, isOutput=False)
    biastrep = nc.declare_dram_parameter(
        "biastrep", [128, 4, G * N], F32R, isOutput=False
    )
    ones8 = nc.declare_dram_parameter("ones8", [128, 4, 8], F16, isOutput=False)
    sel = nc.declare_dram_parameter("sel", [8, 4, 128], F16, isOutput=False)
    ident = nc.declare_dram_parameter("ident", [128, 128], F32R, isOutput=False)
    if with_qkv_bias:
        qkvb = nc.declare_dram_parameter("qkvb", [1, HQKV], F32R, isOutput=False)
    if with_proj_bias:
        projb = nc.declare_dram_parameter("projb", [1, DIM], F32R, isOutput=False)
    outt = nc.declare_dram_parameter("outt", [b_core, 3, 128, N], F32, isOutput=True)

    with tile.TileContext(nc) as tc:
        with (
            tc.tile_pool(name="weights", bufs=1) as wpool,
            tc.tile_pool(name="xin", bufs=2) as xpool,
            tc.tile_pool(name="qk", bufs=2) as qkpool,
            tc.tile_pool(name="vsb", bufs=2) as vpool,
            tc.tile_pool(name="attn", bufs=2) as apool,
            tc.tile_pool(name="av", bufs=2) as avpool,
            tc.tile_pool(name="fin", bufs=2) as fpool,
            tc.tile_pool(name="ps", bufs=8, space="PSUM") as pspool,
        ):
            # ---- resident constants ----
            wt_sb = [
                wpool.tile([128, HQKV], F32R, tag=f"wt{c}", name=f"wt{c}")
                for c in range(3)
            ]
            for c in range(3):
                nc.sync.dma_start(out=wt_sb[c][:], in_=wt[c])
            projt_sb = wpool.tile([128, H, DIM], F32R, tag="projt")
            nc.sync.dma_start(out=projt_sb[:], in_=projt[:])
            biastrep_sb = wpool.tile([128, 4, G * N], F32R, tag="biastrep")
            nc.sync.dma_start(out=biastrep_sb[:], in_=biastrep[:])
            ones8_sb = wpool.tile([128, 4, 8], F16, tag="ones8")
            nc.sync.dma_start(out=ones8_sb[:], in_=ones8[:])
            sel_sb = wpool.tile([8, 4, 128], F16, tag="sel")
            nc.sync.dma_start(out=sel_sb[:], in_=sel[:])
            ident_sb = wpool.tile([128, 128], F32R, tag="ident")
            nc.sync.dma_start(out=ident_sb[:], in_=ident[:])
            zeros_sb = wpool.tile([1, 128], F16, tag="zeros")
            nc.vector.memset(zeros_sb[:], 0.0)
            if with_qkv_bias:
                qkvb_sb = wpool.tile([1, HQKV], F32R, tag="qkvb")
                nc.sync.dma_start(out=qkvb_sb[:], in_=qkvb[:])
            if with_proj_bias:
                projb_sb = wpool.tile([1, DIM], F32R, tag="projb")
                nc.sync.dma_start(out=projb_sb[:], in_=projb[:])
            if with_qkv_bias or with_proj_bias:
                onesw_sb = wpool.tile([1, G * N], F32R, tag="onesw")
                nc.vector.memset(onesw_sb[:], 1.0)

            b0 = 0
            for g, gsz in enumerate(_group_sizes(b_core)):
                W = gsz * N
                npair = (gsz + 1) // 2

                # ---- load x group (transposed+padded on host) ----
                # padded layout (64-stride per batch) for the v-GEMM stationary
                xt_sb = [
                    xpool.tile([128, gsz, NP], F32R, tag=f"xt{c}", name=f"xt{c}_{g}")
                    for c in range(3)
                ]
                # contiguous layout for the qkT moving operand
                xt_cg = [
                    xpool.tile([128, gsz, N], F32R, tag=f"xtc{c}", name=f"xtc{c}_{g}")
                    for c in range(3)
                ]
                for c in range(3):
                    nc.sync.dma_start(
                        out=xt_sb[c][:],
                        in_=xt[b0 : b0 + gsz, c].rearrange("b p m -> p b m"),
                    )
                    nc.sync.dma_start(
                        out=xt_cg[c][:],
                        in_=xt[b0 : b0 + gsz, c, :, :N].rearrange("b p m -> p b m"),
                    )

                # ---- qkT GEMM: psum [128, W] x4 (q heads 0-3, 4-7, k 0-3, 4-7)
                qk_sb = []
                for mc in range(4):
                    ps = pspool.tile([128, 512], F32, tag="ps")
                    for c in range(3):
                        nc.tensor.matmul(
                            ps[:, :W],
                            wt_sb[c][:, mc * 128 : (mc + 1) * 128],
                            xt_cg[c].rearrange("p b m -> p (b m)"),
                            start=(c == 0),
                            stop=(c == 2 and not with_qkv_bias),
                        )
                    if with_qkv_bias:
                        nc.tensor.matmul(
                            ps[:, :W],
                            qkvb_sb[:, mc * 128 : (mc + 1) * 128],
                            onesw_sb[:, :W],
                            start=False,
                            stop=True,
                        )
                    sb = qkpool.tile([128, G * N], F16, tag=f"qk{mc}")
                    nc.vector.tensor_copy(sb[:, :W], ps[:, :W])
                    qk_sb.append(sb)

                # ---- v GEMM: per batch pair, m(2-batch packed) x 1024 feats
                v_sb = vpool.tile([128, G // 2, H, D], F16, tag="vsb")
                xt_flat = [xt_sb[c].rearrange("p b m -> p (b m)") for c in range(3)]
                for j in range(npair):
                    mwidth = 113 if 2 * j + 1 < gsz else 49
                    for half in range(2):
                        ps = pspool.tile([113, 512], F32, tag="ps")
                        for c in range(3):
                            nc.tensor.matmul(
                                ps[:mwidth, :],
                                xt_flat[c][:, j * 2 * NP : j * 2 * NP + mwidth],
                                wt_sb[c][:, 512 + half * 512 : 1024 + half * 512],
                                start=(c == 0),
                                stop=(c == 2 and not with_qkv_bias),
                            )
                        if with_qkv_bias:
                            nc.tensor.matmul(
                                ps[:mwidth, :],
                                onesw_sb[:, :mwidth],
                                qkvb_sb[:, 512 + half * 512 : 1024 + half * 512],
                                start=False,
                                stop=True,
                            )
                        nc.scalar.copy(
                            v_sb[:mwidth, j, half * 4 : half * 4 + 4, :],
                            ps[:mwidth, :],
                        )

                # ---- scoresT + softmax, bank q holds heads {q, q+4} so all
                # writes to one bank come from PE row-tile q*32 ----
                fT = []
                for q in range(4):
                    ps = pspool.tile([128, 512], F32, tag="ps")
                    # seed bias into psum (also fills pad rows with finite vals)
                    nc.tensor.matmul(
                        ps[:, :W],
                        ident_sb[:],
                        biastrep_sb[:, q, :W],
                        start=True,
                        stop=False,
                    )
                    for j in range(npair):
                        for t in range(2):  # head = q + 4*t
                            h = q + 4 * t
                            ktile = qk_sb[2 + t]
                            qtile = qk_sb[t]
                            hbase = q * 32
                            for par in range(2):  # batch parity
                                b = 2 * j + par
                                if b >= gsz:
                                    continue
                                col = (2 * j + t) * N
                                nc.tensor.matmul(
                                    ps[par * 64 : par * 64 + N, col : col + N],
                                    ktile[hbase : hbase + 32, b * N : (b + 1) * N],
                                    qtile[hbase : hbase + 32, b * N : (b + 1) * N],
                                    start=False,
                                    stop=False,
                                    tile_position=(hbase, par * 64),
                                )
                    # close the bank-wide accumulation group; strided columns
                    # overlap every scores sub-region so this schedules last
                    nc.tensor.matmul(
                        bass.AP(
                            tensor=ps.tensor,
                            offset=ps.offset,
                            ap=[ps.ap[0], [N, 2 * npair]],
                        ),
                        zeros_sb[:],
                        zeros_sb[:, : 2 * npair],
                        start=False,
                        stop=True,
                    )
                    f = apool.tile([128, G * N], F16, tag=f"fT{q}")
                    nc.scalar.activation(
                        f[:, :W], ps[:, :W], mybir.ActivationFunctionType.Exp
                    )
                    fT.append(f)

                # denominators: accumulate [8, W] over the 4 banks
                ps_s = pspool.tile([8, 512], F32, tag="ps")
                for q in range(4):
                    nc.tensor.matmul(
                        ps_s[:, :W],
                        ones8_sb[:, q, :],
                        fT[q][:, :W],
                        start=(q == 0),
                        stop=(q == 3),
                    )
                recip = apool.tile([8, G * N], F16, tag="recip")
                with nc.allow_low_precision(reason="softmax denominators in fp16"):
                    nc.vector.reciprocal(recip[:, :W], ps_s[:, :W])

                # normalize: bcast recip over m rows, multiply into attnT
                attnT = []
                for q in range(4):
                    ps_b = pspool.tile([128, 512], F32, tag="ps")
                    nc.tensor.matmul(
                        ps_b[:, :W], sel_sb[:, q, :], recip[:, :W], start=True, stop=True
                    )
                    a = apool.tile([128, G * N], F16, tag=f"attnT{q}")
                    nc.vector.tensor_mul(a[:, :W], fT[q][:, :W], ps_b[:, :W])
                    attnT.append(a)

                # ---- AV: avT_h [128(d), W] per head; separate psum banks per
                # batch parity (writes come from row-tiles 0 and 64) ----
                avh_sb = avpool.tile([128, H, G // 2, 2, N], F32R, tag="avh")
                ne = (gsz + 1) // 2  # number of even batches
                no = gsz // 2  # number of odd batches
                for h in range(H):
                    pse = pspool.tile([128, 512], F32, tag="ps", name=f"avE{g}_{h}")
                    pso = pspool.tile([128, 512], F32, tag="ps", name=f"avO{g}_{h}")
                    for b in range(gsz):
                        par = b % 2
                        j = b // 2
                        col = (2 * j + (h // 4)) * N
                        tgt = pso if par else pse
                        nc.tensor.matmul(
                            tgt[:, j * N : (j + 1) * N],
                            v_sb[par * 64 : par * 64 + N, j, h, :],
                            attnT[h % 4][par * 64 : par * 64 + N, col : col + N],
                            start=(b == par),
                            stop=(b >= gsz - 2),
                        )
                    if h % 2 == 0:
                        nc.vector.tensor_copy(
                            avh_sb[:, h, :ne, 0, :], pse[:, : ne * N]
                        )
                        nc.scalar.copy(avh_sb[:, h, :no, 1, :], pso[:, : no * N])
                    else:
                        nc.scalar.copy(avh_sb[:, h, :ne, 0, :], pse[:, : ne * N])
                        nc.vector.tensor_copy(
                            avh_sb[:, h, :no, 1, :], pso[:, : no * N]
                        )

                # ---- proj: finalT [384, W] over 8 head chunks ----
                for mc in range(3):
                    ps = pspool.tile([128, 512], F32, tag="ps")
                    for h in range(H):
                        nc.tensor.matmul(
                            ps[:, :W],
                            projt_sb[:, h, mc * 128 : (mc + 1) * 128],
                            avh_sb[:, h, :npair, :, :],
                            start=(h == 0),
                            stop=(h == 7 and not with_proj_bias),
                        )
                    if with_proj_bias:
                        nc.tensor.matmul(
                            ps[:, :W],
                            projb_sb[:, mc * 128 : (mc + 1) * 128],
                            onesw_sb[:, :W],
                            start=False,
                            stop=True,
                        )
                    fin = fpool.tile([128, G, N], F32, tag=f"fin{mc}")
                    if mc == 0:
                        nc.vector.tensor_copy(fin[:, :gsz, :], ps[:, :W])
                    else:
                        nc.scalar.copy(fin[:, :gsz, :], ps[:, :W])
                    nc.sync.dma_start(
                        out=outt[b0 : b0 + gsz, mc].rearrange("b p m -> p b m"),
                        in_=fin[:, :gsz, :],
                    )

                b0 += gsz

    nsplit = _legalize_waits(nc)
    if nsplit:
        print(f"[kernel] split {nsplit} excess sem waits onto nops")
    return nc


def _host_prep(x, qkv_w, qkv_b, proj_w, proj_b, attn_bias, bias_idxs):
    """Build per-core input maps."""
    scale = KD ** -0.5
    # reorder qkv weight rows: per head [q(32) k(32) v(128)] -> q_all k_all v_all
    wq = np.concatenate([qkv_w[h * 192 : h * 192 + 32] for h in range(H)], 0) * scale
    wk = np.concatenate([qkv_w[h * 192 + 32 : h * 192 + 64] for h in range(H)], 0)
    wv = np.concatenate([qkv_w[h * 192 + 64 : h * 192 + 192] for h in range(H)], 0)
    w_cat = np.concatenate([wq, wk, wv], 0)  # [1536, 384]
    wT = np.ascontiguousarray(w_cat.T).astype(np.float32)  # [384, 1536]
    wt_arr = wT.reshape(3, 128, HQKV)

    projt_arr = np.ascontiguousarray(proj_w.T).astype(np.float32).reshape(
        128 * H, DIM
    )  # rows ordered (h, d)
    projt_arr = projt_arr.reshape(H, 128, DIM).transpose(1, 0, 2).copy()  # [128,H,DIM]

    bias_full = attn_bias[:, bias_idxs]  # [H, N, N] indexed (h, n, m)
    # biastrep[q]: rows par*64+m, cols (j, hp, n) -> bias[2q+hp, n, m]
    biastrep_arr = np.zeros((128, 4, G * N), np.float32)
    for q in range(4):
        for t in range(2):
            bT = bias_full[q + 4 * t].T  # [m, n]
            for j in range(G // 2):
                for par in range(2):
                    biastrep_arr[par * 64 : par * 64 + N, q, (2 * j + t) * N : (2 * j + t + 1) * N] = bT

    ones8_arr = np.zeros((128, 4, 8), np.float16)
    for q in range(4):
        for par in range(2):
            ones8_arr[par * 64 : par * 64 + N, q, 2 * q + par] = 1.0

    sel_arr = np.zeros((8, 4, 128), np.float16)
    for q in range(4):
        for par in range(2):
            sel_arr[2 * q + par, q, par * 64 : par * 64 + N] = 1.0

    ident_arr = np.eye(128, dtype=np.float32)

    # x: [B, N, DIM] -> transposed, padded [B, 3, 128, NP]
    xT = np.zeros((B, 3, 128, NP), np.float32)
    xT[:, :, :, :N] = (
        x.transpose(0, 2, 1).reshape(B, 3, 128, N).astype(np.float32)
    )

    qb = np.concatenate(
        [qkv_b[h * 192 : h * 192 + 32] for h in range(H)]
    ) * scale
    kb = np.concatenate([qkv_b[h * 192 + 32 : h * 192 + 64] for h in range(H)])
    vb = np.concatenate([qkv_b[h * 192 + 64 : h * 192 + 192] for h in range(H)])
    qkvb_arr = np.concatenate([qb, kb, vb]).astype(np.float32).reshape(1, HQKV)
    projb_arr = proj_b.astype(np.float32).reshape(1, DIM)

    with_qkv_bias = bool(np.any(qkvb_arr))
    with_proj_bias = bool(np.any(projb_arr))

    in_maps = []
    for c in range(N_CORES):
        m = {
            "xt": xT[c * B_CORE : (c + 1) * B_CORE],
            "wt": wt_arr,
            "projt": projt_arr,
            "biastrep": biastrep_arr,
            "ones8": ones8_arr,
            "sel": sel_arr,
            "ident": ident_arr,
        }
        if with_qkv_bias:
            m["qkvb"] = qkvb_arr
        if with_proj_bias:
            m["projb"] = projb_arr
        in_maps.append(m)
    return in_maps, with_qkv_bias, with_proj_bias


def _get_runner(with_qkv_bias, with_proj_bias):
    """Build (once) a reusable jitted SPMD executable, mirroring
    concourse.bass2jax.run_bass_via_pjrt but cached for repeat timing."""
    key = (with_qkv_bias, with_proj_bias)
    if key in _CACHE:
        return _CACHE[key]

    import jax
    from jax.sharding import Mesh, PartitionSpec
    from jax.experimental.shard_map import shard_map
    from concourse import bass2jax
    from concourse.bass2jax import (
        _bass_exec_p,
        install_neuronx_cc_hook,
        partition_id_tensor,
    )

    install_neuronx_cc_hook()
    nc = _build_bass(with_qkv_bias, with_proj_bias)
    partition_name = nc.partition_id_tensor.name if nc.partition_id_tensor else None

    in_names, out_names, out_avals, zero_outs = [], [], [], []
    for alloc in nc.m.functions[0].allocations:
        if not isinstance(alloc, mybir.MemoryLocationSet):
            continue
        name = alloc.memorylocations[0].name
        if alloc.kind == "ExternalInput":
            if name != partition_name:
                in_names.append(name)
        elif alloc.kind == "ExternalOutput":
            shape = tuple(alloc.tensor_shape)
            dtype = mybir.dt.np(alloc.dtype)
            out_names.append(name)
            out_avals.append(jax.core.ShapedArray(shape, dtype))
            zero_outs.append(np.zeros(shape, dtype))
    n_params = len(in_names)
    n_outs = len(out_avals)
    all_names = in_names + out_names
    if partition_name is not None:
        all_names = all_names + [partition_name]
    donate = tuple(range(n_params, n_params + n_outs))

    def _body(*args):
        operands = list(args)
        if partition_name is not None:
            operands.append(partition_id_tensor())
        outs = _bass_exec_p.bind(
            *operands,
            out_avals=tuple(out_avals),
            in_names=tuple(all_names),
            out_names=tuple(out_names),
            lowering_input_output_aliases=(),
            sim_require_finite=True,
            sim_require_nnan=True,
            nc=nc,
        )
        return tuple(outs)

    devices = jax.devices()[:N_CORES]
    mesh = Mesh(np.asarray(devices), ("core",))
    in_specs = (PartitionSpec("core"),) * (n_params + n_outs)
    out_specs = (PartitionSpec("core"),) * n_outs
    sharded = jax.jit(
        shard_map(
            _body, mesh=mesh, in_specs=in_specs, out_specs=out_specs, check_rep=False
        ),
        donate_argnums=donate,
        keep_unused=True,
    )

    runner = {
        "sharded": sharded,
        "in_names": in_names,
        "out_names": out_names,
        "out_avals": out_avals,
        "zero_outs": zero_outs,
    }
    _CACHE[key] = runner
    return runner


def _run_device(in_maps, runner):
    concat_in = [
        np.concatenate([m[name] for m in in_maps], axis=0)
        for name in runner["in_names"]
    ]
    concat_zeros = [
        np.zeros((N_CORES * z.shape[0], *z.shape[1:]), z.dtype)
        for z in runner["zero_outs"]
    ]
    out_arrs = runner["sharded"](*concat_in, *concat_zeros)
    return np.asarray(out_arrs[0])  # [8*B_CORE, 3, 128, 49]


def kernel(**inputs):
    x = np.asarray(inputs["x"], np.float32)
    in_maps, wqb, wpb = _host_prep(
        x,
        np.asarray(inputs["qkv_w"], np.float32),
        np.asarray(inputs["qkv_b"], np.float32),
        np.asarray(inputs["proj_w"], np.float32),
        np.asarray(inputs["proj_b"], np.float32),
        np.asarray(inputs["attn_bias"], np.float32),
        np.asarray(inputs["bias_idxs"]),
    )
    runner = _get_runner(wqb, wpb)
    outt = _run_device(in_maps, runner)  # [B, 3, 128, 49]
    out = outt.reshape(B, DIM, N).transpose(0, 2, 1)
    return np.ascontiguousarray(out)


# revision 24
# speedup vs baseline: 69.4863x; 69.4863x over previous
"""Trainium2 Bass kernel for nn_BasicAttention (dense transformer block).

Strategy (pure data parallel over 8 NeuronCores, batch-sharded):
  per core: B_CORE=256 batches of [N=49, DIM=384].
  Per group of G=10 batches (free-packed width W = G*49):
    1. qkT GEMM   : [512 qk-feats, W]  = wT_qk.T @ xT   (f32r, moving=W)
    2. v GEMM     : [113(2-batch packed m), 512]x2 per batch pair (f32r)
    3. scoresT    : bias pre-seeded into PSUM via identity matmul, then
                    per-(batch,head) fp16 matmuls K=32 packed into PE
                    sub-tiles (auto tile_position from base partitions)
    4. softmax    : exp on ACT (PSUM->SBUF fp16), denominators via
                    ones-matmul on PE, reciprocal on DVE, broadcast of
                    1/s back over m-rows via selector matmul on PE
    5. AV         : fp16 matmuls [49,128]x[49,49] -> avT [128(d), W]
    6. proj       : finalT [384, W] accumulated over 8 head chunks (f32r)
  Host side: weight reordering/transposition, rel-pos bias gather,
  x transposition+padding, output transposition.
"""

import sys

sys.path.insert(0, "/opt/trn_rl_repo")

import numpy as np

import concourse.bass as bass
import concourse.mybir as mybir
import concourse.tile as tile
from concourse.vector_clock import ScopedClock

# ---------------- problem constants (hardcoded per spec) ----------------
B = 2048
N = 49
DIM = 384
H = 8
KD = 32
D = 128
DH = H * D  # 1024
HQKV = 1536
N_CORES = 8
B_CORE = B // N_CORES  # 256
G = 10  # batches per group
NP = 64  # padded token stride (m padded 49->64 inside x tiles)

F32R = mybir.dt.float32r
F32 = mybir.dt.float32
F16 = mybir.dt.float16

_CACHE = {}


# ---------------- Tile drain workaround ----------------
def _patched_drain_and_barrier(self, tick_clock, wait_clock):
    # walrus rejects >1 sem wait on the final SP Drain (TPB_CTRL); spread
    # the global-clock waits across single-wait SP nops instead.
    nc = self.nc
    probe = nc.sync.nop()
    wait_clock.add_sem_waits(probe.ins, ScopedClock({None: tick_clock.global_clock}))
    waits = []
    if probe.ins.sync_info and probe.ins.sync_info.on_wait:
        waits = list(probe.ins.sync_info.on_wait)
        probe.ins.sync_info.on_wait = waits[:1]
    assert self.sems is not None
    handles = list(self.sems.allocated().values())
    for w in waits[1:]:
        n = nc.sync.nop()
        n._wait_ge(handles[0], 0)
        n.ins.sync_info.on_wait = [w]
    nc.sync.drain()
    nc.all_engine_barrier()
    popped = nc._tile_sem_poison_stack.pop()
    assert popped is self._sem_poison
    nc.clear_and_free_semaphores(handles)
    nc.all_engine_barrier()


tile.TileContext._drain_and_barrier = _patched_drain_and_barrier


def _legalize_waits(nc, max_waits=1):
    """walrus on this toolchain rejects instructions carrying more than one
    sem wait; split excess waits onto preceding same-engine nops."""
    import bass_rust

    cnt = 0
    for f in nc.m.functions:
        for b in f.blocks:
            insts = b.instructions
            inserts = []
            for idx, inst in enumerate(insts):
                si = inst.sync_info
                waits = list(si.on_wait) if (si and si.on_wait) else []
                if len(waits) <= max_waits:
                    continue
                nops = []
                for w in waits[max_waits:]:
                    cnt += 1
                    nop = mybir.InstNoOp(
                        name=f"I-waitsplit-{cnt}",
                        engine=inst.engine,
                        ins=[],
                        outs=[],
                        sync_info=bass_rust.SyncInfo(on_wait=[w], on_update=[]),
                    )
                    try:
                        nc.register_instruction(nop)
                    except Exception:
                        pass
                    nops.append(nop)
                si.on_wait = waits[:max_waits]
                inserts.append((idx, nops))
            for idx, nops in reversed(inserts):
                for nop in reversed(nops):
                    insts.insert(idx, nop)
    return cnt


def _group_sizes(b_core=B_CORE):
    sizes = []
    b = 0
    while b < b_core:
        g = min(G, b_core - b)
        sizes.append(g)
        b += g
    return sizes


def _build_bass(with_qkv_bias, with_proj_bias, b_core=B_CORE):
    nc = bass.Bass()

    xt = nc.declare_dram_parameter("xt", [b_core, 3, 128, NP], F32R, isOutput=False)
    wt = nc.declare_dram_parameter("wt", [3, 128, HQKV], F32R, isOutput=False)
    projt = nc.declare_dram_parameter("projt", [128, H, DIM], F32R, isOutput=False)
    biastrep = nc.declare_dram_parameter(
        "biastrep", [128, 4, G * N], F32R, isOutput=False
    )
    ones8 = nc.declare_dram_parameter("ones8", [128, 4, 8], F16, isOutput=False)
    sel = nc.declare_dram_parameter("sel", [8, 4, 128], F16, isOutput=False)
    ident = nc.declare_dram_parameter("ident", [128, 128], F32R, isOutput=False)
    if with_qkv_bias:
        qkvb = nc.declare_dram_parameter("qkvb", [1, HQKV], F32R, isOutput=False)
    if with_proj_bias:
        projb = nc.declare_dram_parameter("projb", [1, DIM], F32R, isOutput=False)
    outt = nc.declare_dram_parameter("outt", [b_core, 3, 128, N], F32, isOutput=True)

    with tile.TileContext(nc) as tc:
        with (
            tc.tile_pool(name="weights", bufs=1) as wpool,
            tc.tile_pool(name="xin", bufs=2) as xpool,
            tc.tile_pool(name="qk", bufs=2) as qkpool,
            tc.tile_pool(name="vsb", bufs=2) as vpool,
            tc.tile_pool(name="attn", bufs=2) as apool,
            tc.tile_pool(name="av", bufs=2) as avpool,
            tc.tile_pool(name="fin", bufs=2) as fpool,
            tc.tile_pool(name="ps", bufs=8, space="PSUM") as pspool,
        ):
            # ---- resident constants ----
            wt_sb = [
                wpool.tile([128, HQKV], F32R, tag=f"wt{c}", name=f"wt{c}")
                for c in range(3)
            ]
            for c in range(3):
                nc.sync.dma_start(out=wt_sb[c][:], in_=wt[c])
            projt_sb = wpool.tile([128, H, DIM], F32R, tag="projt")
            nc.sync.dma_start(out=projt_sb[:], in_=projt[:])
            biastrep_sb = wpool.tile([128, 4, G * N], F32R, tag="biastrep")
            nc.sync.dma_start(out=biastrep_sb[:], in_=biastrep[:])
            ones8_sb = wpool.tile([128, 4, 8], F16, tag="ones8")
            nc.sync.dma_start(out=ones8_sb[:], in_=ones8[:])
            sel_sb = wpool.tile([8, 4, 128], F16, tag="sel")
            nc.sync.dma_start(out=sel_sb[:], in_=sel[:])
            ident_sb = wpool.tile([128, 128], F32R, tag="ident")
            nc.sync.dma_start(out=ident_sb[:], in_=ident[:])
            zeros_sb = wpool.tile([1, 128], F16, tag="zeros")
            nc.vector.memset(zeros_sb[:], 0.0)
            if with_qkv_bias:
                qkvb_sb = wpool.tile([1, HQKV], F32R, tag="qkvb")
                nc.sync.dma_start(out=qkvb_sb[:], in_=qkvb[:])
            if with_proj_bias:
                projb_sb = wpool.tile([1, DIM], F32R, tag="projb")
                nc.sync.dma_start(out=projb_sb[:], in_=projb[:])
            if with_qkv_bias or with_proj_bias:
                onesw_sb = wpool.tile([1, G * N], F32R, tag="onesw")
                nc.vector.memset(onesw_sb[:], 1.0)

            b0 = 0
            for g, gsz in enumerate(_group_sizes(b_core)):
                W = gsz * N
                npair = (gsz + 1) // 2

                # ---- load x group (transposed+padded on host) ----
                # padded layout (64-stride per batch) for the v-GEMM stationary
                xt_sb = [
                    xpool.tile([128, gsz, NP], F32R, tag=f"xt{c}", name=f"xt{c}_{g}")
                    for c in range(3)
                ]
                # contiguous layout for the qkT moving operand
                xt_cg = [
                    xpool.tile([128, gsz, N], F32R, tag=f"xtc{c}", name=f"xtc{c}_{g}")
                    for c in range(3)
                ]
                for c in range(3):
                    nc.sync.dma_start(
                        out=xt_sb[c][:],
                        in_=xt[b0 : b0 + gsz, c].rearrange("b p m -> p b m"),
                    )
                    nc.sync.dma_start(
                        out=xt_cg[c][:],
                        in_=xt[b0 : b0 + gsz, c, :, :N].rearrange("b p m -> p b m"),
                    )

                # ---- qkT GEMM: psum [128, W] x4 (q heads 0-3, 4-7, k 0-3, 4-7)
                qk_sb = []
                for mc in range(4):
                    ps = pspool.tile([128, 512], F32, tag="ps")
                    for c in range(3):
                        nc.tensor.matmul(
                            ps[:, :W],
                            wt_sb[c][:, mc * 128 : (mc + 1) * 128],
                            xt_cg[c].rearrange("p b m -> p (b m)"),
                            start=(c == 0),
                            stop=(c == 2 and not with_qkv_bias),
                        )
                    if with_qkv_bias:
                        nc.tensor.matmul(
                            ps[:, :W],
                            qkvb_sb[:, mc * 128 : (mc + 1) * 128],
                            onesw_sb[:, :W],
                            start=False,
                            stop=True,
                        )
                    sb = qkpool.tile([128, G * N], F16, tag=f"qk{mc}")
                    nc.vector.tensor_copy(sb[:, :W], ps[:, :W])
                    qk_sb.append(sb)

                # ---- v GEMM: per batch pair, m(2-batch packed) x 1024 feats
                v_sb = vpool.tile([128, G // 2, H, D], F16, tag="vsb")
                xt_flat = [xt_sb[c].rearrange("p b m -> p (b m)") for c in range(3)]
                for j in range(npair):
                    mwidth = 113 if 2 * j + 1 < gsz else 49
                    for half in range(2):
                        ps = pspool.tile([113, 512], F32, tag="ps")
                        for c in range(3):
                            nc.tensor.matmul(
                                ps[:mwidth, :],
                                xt_flat[c][:, j * 2 * NP : j * 2 * NP + mwidth],
                                wt_sb[c][:, 512 + half * 512 : 1024 + half * 512],
                                start=(c == 0),
                                stop=(c == 2 and not with_qkv_bias),
                            )
                        if with_qkv_bias:
                            nc.tensor.matmul(
                                ps[:mwidth, :],
                                onesw_sb[:, :mwidth],
                                qkvb_sb[:, 512 + half * 512 : 1024 + half * 512],
                                start=False,
                                stop=True,
                            )
                        nc.scalar.copy(
                            v_sb[:mwidth, j, half * 4 : half * 4 + 4, :],
                            ps[:mwidth, :],
                        )

                # ---- scoresT + softmax, bank q holds heads {q, q+4} so all
                # writes to one bank come from PE row-tile q*32 ----
                fT = []
                for q in range(4):
                    ps = pspool.tile([128, 512], F32, tag="ps")
                    # seed bias into psum (also fills pad rows with finite vals)
                    nc.tensor.matmul(
                        ps[:, :W],
                        ident_sb[:],
                        biastrep_sb[:, q, :W],
                        start=True,
                        stop=False,
                    )
                    for j in range(npair):
                        for t in range(2):  # head = q + 4*t
                            h = q + 4 * t
                            ktile = qk_sb[2 + t]
                            qtile = qk_sb[t]
                            hbase = q * 32
                            for par in range(2):  # batch parity
                                b = 2 * j + par
                                if b >= gsz:
                                    continue
                                col = (2 * j + t) * N
                                nc.tensor.matmul(
                                    ps[par * 64 : par * 64 + N, col : col + N],
                                    ktile[hbase : hbase + 32, b * N : (b + 1) * N],
                                    qtile[hbase : hbase + 32, b * N : (b + 1) * N],
                                    start=False,
                                    stop=False,
                                    tile_position=(hbase, par * 64),
                                )
                    # close the bank-wide accumulation group; strided columns
                    # overlap every scores sub-region so this schedules last
                    nc.tensor.matmul(
                        bass.AP(
                            tensor=ps.tensor,
                            offset=ps.offset,
                            ap=[ps.ap[0], [N, 2 * npair]],
                        ),
                        zeros_sb[:],
                        zeros_sb[:, : 2 * npair],
                        start=False,
                        stop=True,
                    )
                    f = apool.tile([128, G * N], F16, tag=f"fT{q}")
                    nc.scalar.activation(
                        f[:, :W], ps[:, :W], mybir.ActivationFunctionType.Exp
                    )
                    fT.append(f)

                # denominators: accumulate [8, W] over the 4 banks
                ps_s = pspool.tile([8, 512], F32, tag="ps")
                for q in range(4):
                    nc.tensor.matmul(
                        ps_s[:, :W],
                        ones8_sb[:, q, :],
                        fT[q][:, :W],
                        start=(q == 0),
                        stop=(q == 3),
                    )
                recip = apool.tile([8, G * N], F16, tag="recip")
                with nc.allow_low_precision(reason="softmax denominators in fp16"):
                    nc.vector.reciprocal(recip[:, :W], ps_s[:, :W])

                # normalize: bcast recip over m rows, multiply into attnT
                attnT = []
                for q in range(4):
                    ps_b = pspool.tile([128, 512], F32, tag="ps")
                    nc.tensor.matmul(
                        ps_b[:, :W], sel_sb[:, q, :], recip[:, :W], start=True, stop=True
                    )
                    a = apool.tile([128, G * N], F16, tag=f"attnT{q}")
                    nc.vector.tensor_mul(a[:, :W], fT[q][:, :W], ps_b[:, :W])
                    attnT.append(a)

                # ---- AV: avT_h [128(d), W] per head; separate psum banks per
                # batch parity (writes come from row-tiles 0 and 64) ----
                avh_sb = avpool.tile([128, H, G // 2, 2, N], F32R, tag="avh")
                ne = (gsz + 1) // 2  # number of even batches
                no = gsz // 2  # number of odd batches
                for h in range(H):
                    pse = pspool.tile([128, 512], F32, tag="ps", name=f"avE{g}_{h}")
                    pso = pspool.tile([128, 512], F32, tag="ps", name=f"avO{g}_{h}")
                    for b in range(gsz):
                        par = b % 2
                        j = b // 2
                        col = (2 * j + (h // 4)) * N
                        tgt = pso if par else pse
                        nc.tensor.matmul(
                            tgt[:, j * N : (j + 1) * N],
                            v_sb[par * 64 : par * 64 + N, j, h, :],
                            attnT[h % 4][par * 64 : par * 64 + N, col : col + N],
                            start=(b == par),
                            stop=(b >= gsz - 2),
                        )
                    if h % 2 == 0:
                        nc.vector.tensor_copy(
                            avh_sb[:, h, :ne, 0, :], pse[:, : ne * N]
                        )
                        nc.scalar.copy(avh_sb[:, h, :no, 1, :], pso[:, : no * N])
                    else:
                        nc.scalar.copy(avh_sb[:, h, :ne, 0, :], pse[:, : ne * N])
                        nc.vector.tensor_copy(
                            avh_sb[:, h, :no, 1, :], pso[:, : no * N]
                        )

                # ---- proj: finalT [384, W] over 8 head chunks ----
                for mc in range(3):
                    ps = pspool.tile([128, 512], F32, tag="ps")
                    for h in range(H):
                        nc.tensor.matmul(
                            ps[:, :W],
                            projt_sb[:, h, mc * 128 : (mc + 1) * 128],
                            avh_sb[:, h, :npair, :, :],
                            start=(h == 0),
                            stop=(h == 7 and not with_proj_bias),
                        )
                    if with_proj_bias:
                        nc.tensor.matmul(
                            ps[:, :W],
                            projb_sb[:, mc * 128 : (mc + 1) * 128],
                            onesw_sb[:, :W],
                            start=False,
                            stop=True,
                        )
                    fin = fpool.tile([128, G, N], F32, tag=f"fin{mc}")
                    if mc == 0:
                        nc.vector.tensor_copy(fin[:, :gsz, :], ps[:, :W])
                    else:
                        nc.scalar.copy(fin[:, :gsz, :], ps[:, :W])
                    nc.sync.dma_start(
                        out=outt[b0 : b0 + gsz, mc].rearrange("b p m -> p b m"),
                        in_=fin[:, :gsz, :],
                    )

                b0 += gsz

    nsplit = _legalize_waits(nc)
    if nsplit:
        print(f"[kernel] split {nsplit} excess sem waits onto nops")
    return nc


def _host_prep(x, qkv_w, qkv_b, proj_w, proj_b, attn_bias, bias_idxs):
    """Build per-core input maps."""
    scale = KD ** -0.5
    # reorder qkv weight rows: per head [q(32) k(32) v(128)] -> q_all k_all v_all
    wq = np.concatenate([qkv_w[h * 192 : h * 192 + 32] for h in range(H)], 0) * scale
    wk = np.concatenate([qkv_w[h * 192 + 32 : h * 192 + 64] for h in range(H)], 0)
    wv = np.concatenate([qkv_w[h * 192 + 64 : h * 192 + 192] for h in range(H)], 0)
    w_cat = np.concatenate([wq, wk, wv], 0)  # [1536, 384]
    wT = np.ascontiguousarray(w_cat.T).astype(np.float32)  # [384, 1536]
    wt_arr = wT.reshape(3, 128, HQKV)

    projt_arr = np.ascontiguousarray(proj_w.T).astype(np.float32).reshape(
        128 * H, DIM
    )  # rows ordered (h, d)
    projt_arr = projt_arr.reshape(H, 128, DIM).transpose(1, 0, 2).copy()  # [128,H,DIM]

    bias_full = attn_bias[:, bias_idxs]  # [H, N, N] indexed (h, n, m)
    # biastrep[q]: rows par*64+m, cols (j, hp, n) -> bias[2q+hp, n, m]
    biastrep_arr = np.zeros((128, 4, G * N), np.float32)
    for q in range(4):
        for t in range(2):
            bT = bias_full[q + 4 * t].T  # [m, n]
            for j in range(G // 2):
                for par in range(2):
                    biastrep_arr[par * 64 : par * 64 + N, q, (2 * j + t) * N : (2 * j + t + 1) * N] = bT

    ones8_arr = np.zeros((128, 4, 8), np.float16)
    for q in range(4):
        for par in range(2):
            ones8_arr[par * 64 : par * 64 + N, q, 2 * q + par] = 1.0

    sel_arr = np.zeros((8, 4, 128), np.float16)
    for q in range(4):
        for par in range(2):
            sel_arr[2 * q + par, q, par * 64 : par * 64 + N] = 1.0

    ident_arr = np.eye(128, dtype=np.float32)

    # x: [B, N, DIM] -> transposed, padded [B, 3, 128, NP]
    xT = np.zeros((B, 3, 128, NP), np.float32)
    xT[:, :, :, :N] = (
        x.transpose(0, 2, 1).reshape(B, 3, 128, N).astype(np.float32)
    )

    qb = np.concatenate(
        [qkv_b[h * 192 : h * 192 + 32] for h in range(H)]
    ) * scale
    kb = np.concatenate([qkv_b[h * 192 + 32 : h * 192 + 64] for h in range(H)])
    vb = np.concatenate([qkv_b[h * 192 + 64 : h * 192 + 192] for h in range(H)])
    qkvb_arr = np.concatenate([qb, kb, vb]).astype(np.float32).reshape(1, HQKV)
    projb_arr = proj_b.astype(np.float32).reshape(1, DIM)

    with_qkv_bias = bool(np.any(qkvb_arr))
    with_proj_bias = bool(np.any(projb_arr))

    in_maps = []
    for c in range(N_CORES):
        m = {
            "xt": xT[c * B_CORE : (c + 1) * B_CORE],
            "wt": wt_arr,
            "projt": projt_arr,
            "biastrep": biastrep_arr,
            "ones8": ones8_arr,
            "sel": sel_arr,
            "ident": ident_arr,
        }
        if with_qkv_bias:
            m["qkvb"] = qkvb_arr
        if with_proj_bias:
            m["projb"] = projb_arr
        in_maps.append(m)
    return in_maps, with_qkv_bias, with_proj_bias


def _get_runner(with_qkv_bias, with_proj_bias):
    """Build (once) a reusable jitted SPMD executable, mirroring
    concourse.bass2jax.run_bass_via_pjrt but cached for repeat timing."""
    key = (with_qkv_bias, with_proj_bias)
    if key in _CACHE:
        return _CACHE[key]

    import jax
    from jax.sharding import Mesh, PartitionSpec
    from jax.experimental.shard_map import shard_map
    from concourse import bass2jax
    from concourse.bass2jax import (
        _bass_exec_p,
        install_neuronx_cc_hook,
        partition_id_tensor,
    )

    install_neuronx_cc_hook()
    nc = _build_bass(with_qkv_bias, with_proj_bias)
    partition_name = nc.partition_id_tensor.name if nc.partition_id_tensor else None

    in_names, out_names, out_avals, zero_outs = [], [], [], []
    for alloc in nc.m.functions[0].allocations:
        if not isinstance(alloc, mybir.MemoryLocationSet):
            continue
        name = alloc.memorylocations[0].name
        if alloc.kind == "ExternalInput":
            if name != partition_name:
                in_names.append(name)
        elif alloc.kind == "ExternalOutput":
            shape = tuple(alloc.tensor_shape)
            dtype = mybir.dt.np(alloc.dtype)
            out_names.append(name)
            out_avals.append(jax.core.ShapedArray(shape, dtype))
            zero_outs.append(np.zeros(shape, dtype))
    n_params = len(in_names)
    n_outs = len(out_avals)
    all_names = in_names + out_names
    if partition_name is not None:
        all_names = all_names + [partition_name]
    donate = tuple(range(n_params, n_params + n_outs))

    def _body(*args):
        operands = list(args)
        if partition_name is not None:
            operands.append(partition_id_tensor())
        outs = _bass_exec_p.bind(
            *operands,
            out_avals=tuple(out_avals),
            in_names=tuple(all_names),
            out_names=tuple(out_names),
            lowering_input_output_aliases=(),
            sim_require_finite=True,
            sim_require_nnan=True,
            nc=nc,
        )
        return tuple(outs)

    devices = jax.devices()[:N_CORES]
    mesh = Mesh(np.asarray(devices), ("core",))
    in_specs = (PartitionSpec("core"),) * (n_params + n_outs)
    out_specs = (PartitionSpec("core"),) * n_outs
    sharded = jax.jit(
        shard_map(
            _body, mesh=mesh, in_specs=in_specs, out_specs=out_specs, check_rep=False
        ),
        keep_unused=True,
    )

    from jax.sharding import NamedSharding

    def stage(concat_arrays):
        """device_put the concatenated inputs + zero out-buffers once."""
        sh = NamedSharding(mesh, PartitionSpec("core"))
        staged = [jax.device_put(a, sh) for a in concat_arrays]
        zeros = [
            jax.device_put(
                np.zeros((N_CORES * z.shape[0], *z.shape[1:]), z.dtype), sh
            )
            for z in zero_outs
        ]
        return staged + zeros

    runner = {
        "sharded": sharded,
        "stage": stage,
        "in_names": in_names,
        "out_names": out_names,
        "out_avals": out_avals,
        "zero_outs": zero_outs,
    }
    _CACHE[key] = runner
    return runner


def _run_device(in_maps, runner):
    concat_in = [
        np.concatenate([m[name] for m in in_maps], axis=0)
        for name in runner["in_names"]
    ]
    staged = runner["stage"](concat_in)
    out_arrs = runner["sharded"](*staged)
    return np.asarray(out_arrs[0])  # [8*B_CORE, 3, 128, 49]


def kernel(**inputs):
    x = np.asarray(inputs["x"], np.float32)
    in_maps, wqb, wpb = _host_prep(
        x,
        np.asarray(inputs["qkv_w"], np.float32),
        np.asarray(inputs["qkv_b"], np.float32),
        np.asarray(inputs["proj_w"], np.float32),
        np.asarray(inputs["proj_b"], np.float32),
        np.asarray(inputs["attn_bias"], np.float32),
        np.asarray(inputs["bias_idxs"]),
    )
    runner = _get_runner(wqb, wpb)
    outt = _run_device(in_maps, runner)  # [B, 3, 128, 49]
    out = outt.reshape(B, DIM, N).transpose(0, 2, 1)
    return np.ascontiguousarray(out)
